# revision 15
# baseline (speedup 1.0000x reference)
"""Contrastive loss kernel for 8 Trainium2 NeuronCores — symmetric v2.

Math (reference):
    s = cosine similarity matrix of x [8192, 256]
    d_i = sum_j exp(s_ij * m_ij / tau)   (m zeroes the diagonal -> diag term = 1)
    v_i = s[i, i^1]                      (adjacent-row positive pairs)
    loss = mean(log d_i - v_i / tau)

Distribution (SPMD-uniform symmetric scheme at 512-row block granularity):
    16 blocks of 512 rows; core c owns blocks {2c, 2c+1} (its 1024 rows).
    Each 512-block computes s against blocks at distances 0..8 (mod 16):
    d0 = its own block (full, covers both pair directions), d1..d7 computed
    once (mirror contribution recovered via column sums), d8 computed by BOTH
    endpoints (each side only keeps its own row sums) -> only +2.8% extra
    work vs the perfect triangle, and every core runs the IDENTICAL program
    on column-rotated data (host un-rotates).

Per core, per m-tile m=0..7 (128 rows each), rotated col window:
    m<4 : cols 0..4607    (own block at [0,512))
    m>=4: cols 512..5119  (own block at [512,1024))
    - fp8e4 DoubleRow matmuls: contraction 256 in ONE pass (x scaled by 8 on
      host; exp scale compensates by 1/64)
    - ACT exp + fused row-sum (accum_out), bf16 exp tiles to SBUF
    - diag/pair logits extracted from the d0 exp tile via mask
      tensor_tensor_reduce on DVE
    - mirror tiles (d1..d7) folded by DVE adds into FA (m<4, rotated cols
      [512,4096)) / FB (m>=4, [1024,4608)) bf16 accumulators, shipped to the
      host, which column-sums and scatters them into d.
"""

import os
import sys

import numpy as np

sys.path.insert(0, "/opt/trn_rl_repo")

import concourse.bass as bass
import concourse.tile as tile
from concourse import mybir
from concourse.bass_utils import run_bass_kernel_spmd

TAU = 0.1
N = 8192
D = 256
P = 128
NCORES = 8
MT = 8                      # m-tiles (128 rows) per core
WIN = 4608                  # columns computed per m-tile (d0..d8)
XCOLS = 5120                # rotated columns staged in SBUF
SCALE8 = 8.0                # host pre-scale of normalized rows before fp8
ACT_SCALE = 1.0 / (SCALE8 * SCALE8 * TAU)
CHUNKS = (2048, 2048, 512)  # per-m-tile PSUM/ACT chunk widths
FLD = 3584                  # fold accumulator width (7 x 512)
# Schraudolph exp-via-int constants (d8 chunk on DVE): exp(ACT_SCALE*x) ~=
# bitcast_f32(int32(x*SCH_A + SCH_B))
SCH_A = float((1 << 23) * 0.15625 * 1.4426950408889634)
SCH_B = float((127.0 - 0.0435) * (1 << 23))
FP32 = mybir.dt.float32
BF16 = mybir.dt.bfloat16
FP8 = mybir.dt.float8e4
DR = mybir.MatmulPerfMode.DoubleRow

_CACHE = {}


def build_nc(repeat=1):
    nc = bass.Bass(trn_type="TRN2")
    xt_d = nc.declare_dram_parameter("xt", [P, 2, XCOLS], FP8, isOutput=False)
    eye_d = nc.declare_dram_parameter("eye", [P, P], BF16, isOutput=False)
    pm_d = nc.declare_dram_parameter("pm", [P, P], BF16, isOutput=False)
    acc_d = nc.declare_dram_parameter("acc", [P, MT * 3], FP32, isOutput=True)
    gd_d = nc.declare_dram_parameter("gd", [P, MT * P], BF16, isOutput=True)
    fa_d = nc.declare_dram_parameter("fa", [P, FLD], BF16, isOutput=True)
    fb_d = nc.declare_dram_parameter("fb", [P, FLD], BF16, isOutput=True)

    with tile.TileContext(nc) as tc:
        with (
            tc.tile_pool(name="big", bufs=2) as big,
            tc.tile_pool(name="small", bufs=1) as small,
            tc.tile_pool(name="scratch", bufs=4) as sc,
            tc.tile_pool(name="psum", bufs=2, space="PSUM") as pp,
        ):
            eye = small.tile([P, P], BF16, tag="eye")
            pm = small.tile([P, P], BF16, tag="pm")
            acc_sb = small.tile([P, MT * 3], FP32, tag="accsb")
            fa = small.tile([P, FLD], BF16, tag="fa")
            fb = small.tile([P, FLD], BF16, tag="fb")

            nc.sync.dma_start(out=eye, in_=eye_d[:, :])
            nc.sync.dma_start(out=pm, in_=pm_d[:, :])
            # Warmup: DVE/ACT observe the mask DMAs; ACT loads the Exp table
            # (~2.7us) off the critical path.
            warm_v = small.tile([P, 1], FP32, tag="warm_v")
            warm_v2 = small.tile([P, 1], FP32, tag="warm_v2")
            warm_a = small.tile([P, P], FP32, tag="warm_a")
            warm_s = small.tile([P, 1], FP32, tag="warm_s")
            nc.vector.reduce_sum(warm_v, eye, axis=mybir.AxisListType.X)
            nc.vector.reduce_sum(warm_v2, pm, axis=mybir.AxisListType.X)
            nc.scalar.activation(out=warm_a, in_=pm,
                                 func=mybir.ActivationFunctionType.Exp,
                                 scale=1.0, accum_out=warm_s)

            import contextlib
            loop_ctx = (tc.For_i(0, repeat, 1)
                        if repeat > 1 else contextlib.nullcontext())
            with loop_ctx:
                _compute_body(nc, tc, sc, pp, small, big, xt_d,
                              eye, pm, acc_sb, fa, fb,
                              acc_d, gd_d, fa_d, fb_d)
    _split_multi_waits(nc)
    return nc


def _compute_body(nc, tc, sc, pp, small, big, xt_d,
                  eye, pm, acc_sb, fa, fb,
                  acc_d, gd_d, fa_d, fb_d):
    if os.environ.get("KERNEL_NULL", "0") == "1":
        nc.vector.memset(acc_sb, 0.0)
        nc.sync.dma_start(out=acc_d[:, :], in_=acc_sb)
        return
    pe_only = os.environ.get("KERNEL_PE_ONLY", "0") == "1"
    no_fold = os.environ.get("KERNEL_NO_FOLD", "0") == "1"
    dma_only = os.environ.get("KERNEL_DMA_ONLY", "0") == "1"
    no_indma = os.environ.get("KERNEL_NO_INDMA", "0") == "1"
    d8dve = os.environ.get("KERNEL_D8DVE", "1") == "1"

    xt = big.tile([P, 2, XCOLS], FP8, tag="xt")
    # input DMA in 1024-col chunks (256KB each) so the first matmuls can
    # start early; ascending order matches consumption order.
    DMA_C = int(os.environ.get("KERNEL_DMA_C", "1024"))
    if not no_indma:
        for c_ in range(XCOLS // DMA_C):
            cs = slice(c_ * DMA_C, (c_ + 1) * DMA_C)
            nc.sync.dma_start(out=xt[:, :, cs], in_=xt_d[:, :, cs])
    if dma_only:
        nc.vector.memset(acc_sb, 0.0)
        nc.sync.dma_start(out=acc_d[:, :], in_=acc_sb)
        return

    # PE warmup against the HAM cold clock (also absorbs mask-DMA sems).
    ps_warm = pp.tile([P, 2048], FP32, tag="super")
    for _w in range(12):
        nc.tensor.matmul(ps_warm[:, 0:P], eye, eye, start=True, stop=True)

    for m in range(MT):
        colbase = 0 if m < 4 else 512
        lhsT = xt[:, :, m * P:(m + 1) * P]
        coff = colbase
        for k, w in enumerate(CHUNKS):
            ps = pp.tile([P, 2048], FP32, tag="super")
            for sub in range(w // 512):
                c0 = coff + sub * 512
                nc.tensor.matmul(ps[:, sub * 512:(sub + 1) * 512],
                                 lhsT, xt[:, :, c0:c0 + 512],
                                 start=True, stop=True, perf_mode=DR)
            if pe_only:
                coff += w
                continue
            if k == 2 and d8dve:
                # d8 chunk: Schraudolph exp + rowsum on DVE, freeing ACT
                si = sc.tile([P, 512], FP32, tag="srexp")
                nc.vector.tensor_scalar(
                    out=si.bitcast(mybir.dt.int32), in0=ps[:, :512],
                    scalar1=SCH_A, scalar2=SCH_B,
                    op0=mybir.AluOpType.mult, op1=mybir.AluOpType.add)
                nc.vector.reduce_sum(
                    acc_sb[:, m * 3 + 2:m * 3 + 3], si,
                    axis=mybir.AxisListType.X)
                coff += w
                continue
            eo = sc.tile([P, 2048], BF16,
                         tag="expout0" if k == 0 else "expout")
            nc.scalar.activation(
                out=eo[:, :w], in_=ps[:, :w],
                func=mybir.ActivationFunctionType.Exp, scale=ACT_SCALE,
                accum_out=acc_sb[:, m * 3 + k:m * 3 + k + 1])
            if no_fold:
                coff += w
                continue
            if k == 0:
                # d0 block shipped to host for diag/pair extraction
                off = m * P - colbase
                gblk = eo[:, off:off + P]
                nc.sync.dma_start(out=gd_d[:, m * P:(m + 1) * P], in_=gblk)
                # fold part of chunk0: rotated [colbase+512, 2048+colbase*?)
                # m<4:  cols [512,2048)  -> FA[0:1536)
                # m>=4: cols [1024,2560) -> FB[0:1536)
                F = fa if m < 4 else fb
                src = eo[:, 512:2048]
                dst = F[:, 0:1536]
                first = m == 0 or m == 4
                if first:
                    nc.vector.tensor_copy(dst, src)
                else:
                    nc.vector.tensor_tensor(out=dst, in0=src, in1=dst,
                                            op=mybir.AluOpType.add)
            elif k == 1:
                # full chunk1 is mirror region:
                # m<4:  cols [2048,4096) -> FA[1536:3584)
                # m>=4: cols [2560,4608) -> FB[1536:3584)
                F = fa if m < 4 else fb
                src = eo[:, 0:2048]
                dst = F[:, 1536:3584]
                first = m == 0 or m == 4
                if first:
                    nc.vector.tensor_copy(dst, src)
                else:
                    nc.vector.tensor_tensor(out=dst, in0=src, in1=dst,
                                            op=mybir.AluOpType.add)
            # k == 2 (d8 both-compute tile): row sums only, no fold
            coff += w
        if m == 3 and not pe_only and not no_fold:
            nc.sync.dma_start(out=fa_d[:, :], in_=fa)

    if pe_only or no_fold:
        if pe_only:
            nc.vector.memset(acc_sb, 0.0)
        nc.vector.memset(fa, 0.0)
        nc.vector.memset(fb, 0.0)
        nc.sync.dma_start(out=fa_d[:, :], in_=fa)
    nc.sync.dma_start(out=fb_d[:, :], in_=fb)
    nc.sync.dma_start(out=acc_d[:, :], in_=acc_sb)


def _split_multi_waits(nc):
    """walrus codegen accepts at most ONE semaphore wait per engine
    instruction; hoist all but the last wait into standalone
    InstEventSemaphore sequencer ops right before it."""
    n_split = 0
    for blk in nc.m.functions[0].blocks:
        new_insts = []
        for inst in blk.instructions:
            si = inst.sync_info
            tname = type(inst).__name__
            if si is not None and len(si.on_wait) > 1 and tname != "InstEventSemaphore":
                waits = list(si.on_wait)
                for j, w in enumerate(waits[:-1]):
                    es = mybir.InstEventSemaphore(
                        name=f"W-split-{inst.name}-{j}")
                    es.engine = inst.engine
                    es.sync_info = mybir.SyncInfo(on_wait=[w], on_update=[])
                    new_insts.append(es)
                    nc.register_instruction(es)
                    n_split += 1
                inst.sync_info = mybir.SyncInfo(
                    on_wait=[waits[-1]], on_update=list(si.on_update))
            new_insts.append(inst)
        blk.instructions[:] = new_insts
    return n_split


def _masks():
    import ml_dtypes
    mdt = ml_dtypes.bfloat16
    eye = np.eye(P, dtype=mdt)
    pm = np.zeros((P, P), dtype=mdt)
    idx = np.arange(P)
    pm[idx, idx ^ 1] = mdt(1.0)
    return eye, pm


def _prepare_inputs(x):
    import ml_dtypes
    x = np.ascontiguousarray(np.asarray(x, dtype=np.float32))
    inv = 1.0 / np.sqrt((x * x).sum(axis=1))
    xn8 = x * (inv * SCALE8)[:, None].astype(np.float32)
    xq = xn8.astype(ml_dtypes.float8_e4m3)          # [N, D]
    eye, pm = _masks()
    in_maps = []
    for c in range(NCORES):
        cols = (np.arange(XCOLS) + c * (N // NCORES)) % N
        xr = xq[cols]                                # [XCOLS, D]
        xt = np.ascontiguousarray(
            xr.T.reshape(2, P, XCOLS).transpose(1, 0, 2))  # [P, 2, XCOLS]
        in_maps.append({"xt": xt, "eye": eye, "pm": pm})
    return in_maps


def _combine(results):
    d = np.zeros(N, dtype=np.float64)
    diag = np.zeros(N, dtype=np.float64)
    pair = np.zeros(N, dtype=np.float64)
    idx = np.arange(P)
    for c in range(NCORES):
        r = results[c]
        acc = np.asarray(r["acc"], dtype=np.float64)    # [128, 24]
        gd = np.asarray(r["gd"], dtype=np.float64)      # [128, 1024]
        fa = np.asarray(r["fa"], dtype=np.float64)      # [128, 3584]
        fb = np.asarray(r["fb"], dtype=np.float64)      # [128, 3584]
        base = c * (N // NCORES)
        for m in range(MT):
            rows = base + m * P + idx
            d[rows] += acc[:, 3 * m:3 * m + 3].sum(axis=1)
            g = gd[:, m * P:(m + 1) * P]
            diag[rows] = g[idx, idx]
            pair[rows] = g[idx, idx ^ 1]
        for F, fbase in ((fa, 512), (fb, 1024)):
            cs = F.sum(axis=0)
            gg = (np.arange(fbase, fbase + FLD) + base) % N
            np.add.at(d, gg, cs)
    d = d - diag + 1.0
    loss = (np.log(d) - np.log(pair)).sum() / N
    return np.float32(loss)


def kernel(x, repeat=None):
    if repeat is None:
        repeat = int(os.environ.get("KERNEL_REPEAT", "1"))
    key = f"nc{repeat}"
    if key not in _CACHE:
        _CACHE[key] = build_nc(repeat)
    nc = _CACHE[key]
    in_maps = _prepare_inputs(x)
    trace = bool(int(os.environ.get("KERNEL_TRACE", "0")))
    res = run_bass_kernel_spmd(nc, in_maps, list(range(NCORES)), trace=trace)
    _CACHE["last_results"] = res
    return _combine(res.results)
